# revision 14
# baseline (speedup 1.0000x reference)
"""Trainium2 Bass kernel for quantized dense layer with Hadamard rotations.

Math (see reference): y = (H2 @ (sq(H2@x) @ sq(w@H1)) @ H1)/(64*64) + bias,
where sq() is per-tensor symmetric int8 stochastic quantization.

Structure: Sylvester Hadamards factor as Kronecker products
(H4096 = H32 (x) H128).  The forward transform per side is a per-128-tile
fp16 PE matmul against H128 (inputs converted fp32->fp16; validated to
cause ~1.2% stochastic-rounding flips = ~0.45% operand error) plus a
cross-tile DVE butterfly in fp16.  Quantized values (<=127) are exact in
fp16, so the core GEMM runs fp16 x fp16 -> fp32 PSUM exactly.  Stochastic
rounding is rint(x*scale + (0.5 - noise)) via the fp32->int16
round-to-nearest cast, narrowed to int8 for the collectives.

Sharding (8 cores): the IN axis is split 8 ways for forward transforms +
quantization.  v2 schedule: the W side runs FIRST so its scale AllReduce
and the AllToAll land early; the CC stream order is
warmup-AR, AR-w, AR-x, A2A(w int8), AG1(x evens int8), AG2(x odds int8),
so the core GEMM starts as soon as AG1 lands instead of waiting for the
whole chain.  All data collectives ship int8 (half the bytes of fp16);
int8->fp16 conversion rides the scalar/vector engines during the
PE-bound GEMM phase.

The GEMM runs in two rounds: evens k-tiles accumulate while AG2 is in
flight and are stashed to SBUF as alpha-scaled fp16; the odds round adds
the stash back via a fused scalar_tensor_tensor.  The inverse fuses the
batch H128 into the post-GEMM PE transpose and applies the batch H4
(bo bits 0-1) as a DVE roll; the feature H128 is one PE matmul per
f-tile.  The remaining inverse factors (feature H32 over core x f-tile,
batch H8 over bo-chunk) fold into the host-side unshard combine.

Known hardware behaviors factored in: PE HAM throttle (1.2 GHz cold /
2.4 GHz after ~3.4us sustained); fp32 matmuls lower to 2 half-speed
passes -- avoid; collectives serialize on one CC stream (emission order
= stream order) with ~10-30us latency floors; a warmup AllReduce at t=0
absorbs the CC-entry barrier and inter-core launch skew; scalar-engine
copies offload PSUM evacuation.
"""
import sys, os
sys.path.insert(0, '/opt/trn_rl_repo')
import numpy as np

B, IN, F = 4096, 2048, 4096
NCORES = 8
CS = IN // NCORES      # 256  per-core IN slice
FS = F // NCORES       # 512  per-core feature block
BT = B // 128          # 32   batch tiles
KT = IN // 128         # 16   contraction tiles
QMAX = 127.0

_cache = {}


def _sylvester(n):
    h = np.array([[1.0]], dtype=np.float32)
    while h.shape[0] < n:
        h = np.block([[h, h], [h, -h]])
    return h


def _build():
    from concourse import bass, bacc, tile, mybir
    import concourse.bass_isa as bass_isa

    DT = mybir.dt.float32
    FH = mybir.dt.float16
    I16 = mybir.dt.int16
    I8 = mybir.dt.int8
    A = mybir.AluOpType
    nph = np.float16

    nc = bacc.Bacc("TRN2", target_bir_lowering=False, debug=False,
                   num_devices=NCORES)

    xk = nc.dram_tensor("xk", [B, CS], FH, kind="ExternalInput")
    nk = nc.dram_tensor("nk", [CS, B], FH, kind="ExternalInput")   # (0.5-noise_x)^T
    wk = nc.dram_tensor("wk", [F, CS], FH, kind="ExternalInput")   # w slice^T
    mk = nc.dram_tensor("mk", [CS, F], FH, kind="ExternalInput")   # 0.5-noise_w
    out = nc.dram_tensor("out", [FS, B], FH, kind="ExternalOutput")

    wu_i = nc.dram_tensor("wu_i", [1, 1], DT)
    wu_o = nc.dram_tensor("wu_o", [1, 1], DT, addr_space="Shared")
    s2_i = nc.dram_tensor("s2_i", [1, 2], DT)
    s2_o = nc.dram_tensor("s2_o", [1, 2], DT, addr_space="Shared")
    xqc0 = nc.dram_tensor("xqc0", [128, B], I8)                    # xq^T k-half 0
    xqc1 = nc.dram_tensor("xqc1", [128, B], I8)                    # xq^T k-half 1
    xqg0 = nc.dram_tensor("xqg0", [NCORES * 128, B], I8, addr_space="Shared")
    xqg1 = nc.dram_tensor("xqg1", [NCORES * 128, B], I8, addr_space="Shared")
    wac = nc.dram_tensor("wac", [IN, FS], I8)                      # A2A contrib
    wblk = nc.dram_tensor("wblk", [IN, FS], I8)

    h128b_d = nc.inline_tensor(_sylvester(128).astype(nph), name="h128b")
    h128n_d = nc.inline_tensor((-_sylvester(128)).astype(nph), name="h128n")
    idb_d = nc.inline_tensor(np.eye(128, dtype=nph), name="idb")
    H4 = _sylvester(4)
    rg = [list(range(NCORES))]

    NB = 32 * CS  # 8192 free columns in a fwd big tile

    def butterfly(nc, bufs, T, blk0, A):
        """FWHT across the tile-index axis of big tensors [128, T*blk0]."""
        n = T.bit_length() - 1
        for s in range(n):
            cur, nxt = bufs(s)
            blk = blk0 << s
            hi = T >> (s + 1)
            for h in range(hi):
                a0 = h * 2 * blk
                a1 = a0 + blk
                nc.vector.tensor_tensor(nxt[:, a0:a0 + blk],
                                        cur[:, a0:a0 + blk],
                                        cur[:, a1:a1 + blk], op=A.add)
                nc.vector.tensor_tensor(nxt[:, a1:a1 + blk],
                                        cur[:, a0:a0 + blk],
                                        cur[:, a1:a1 + blk],
                                        op=A.subtract)

    with tile.TileContext(nc) as tc:
        with tc.tile_pool(name="consts", bufs=1) as cpool:
            h128b = cpool.tile([128, 128], FH)
            h128n = cpool.tile([128, 128], FH)
            idb = cpool.tile([128, 128], FH)
            nc.sync.dma_start(h128b[:], h128b_d[:])
            nc.sync.dma_start(h128n[:], h128n_d[:])
            nc.sync.dma_start(idb[:], idb_d[:])
            qsc = tc.alloc_tile_pool(name="qsc", bufs=1)
            wu = qsc.tile([1, 1], DT, tag="wu", name="wu")
            nc.vector.memset(wu[0:1, 0:1], 0.0)
            nc.sync.dma_start(wu_i[:], wu[0:1, 0:1])
            nc.gpsimd.collective_compute(
                "AllReduce", A.max, replica_groups=rg,
                ins=[wu_i.ap().opt()], outs=[wu_o.ap().opt()])

            # ================= forward transforms + quant =================
            with tc.tile_pool(name="fwd", bufs=2) as fp_, \
                 tc.tile_pool(name="fin", bufs=4) as fin, \
                 tc.tile_pool(name="fps", bufs=1, space="PSUM") as fps, \
                 tc.tile_pool(name="qtmp", bufs=2) as qtmp, \
                 tc.tile_pool(name="qT", bufs=3) as qTp:

                def fwd_side(src_tile_ap, ntiles, side):
                    am2 = qsc.tile([128, 2], DT, tag=f"am{side}",
                                   name=f"am{side}")
                    fwd_side.am2 = am2
                    fwd_side.red1 = qsc.tile([1, 1], DT, tag=f"r1{side}",
                                             name=f"r1{side}")
                    bigA = fp_.tile([128, NB], FH, tag="bigA",
                                    name=f"bigA{side}")
                    bigB = fp_.tile([128, NB], FH, tag="bigB",
                                    name=f"bigB{side}")
                    # H128 (x) H4: per 4-tile group, each output tile is a
                    # 4-term +/-H128 PSUM accumulation (DVE TT runs at 1x
                    # mode, so trading 2 butterfly stages for PE matmuls
                    # wins; the PE load also warms the HAM clock early)
                    for g4 in range(ntiles // 4):
                        thg = fin.tile([128, 4 * CS], FH, tag="finh",
                                       name="finth", bufs=4)
                        nc.sync.dma_start(
                            thg[:].rearrange("p (m c) -> p m c", m=4),
                            src_tile_ap(g4))
                        ths = [thg[:, m * CS:(m + 1) * CS] for m in range(4)]
                        for mp in range(4):
                            o = g4 * 4 + mp
                            ps = fps.tile([128, CS], DT, tag="ps",
                                          name="fpst", bufs=4)
                            for m in range(4):
                                st = h128b if H4[mp, m] > 0 else h128n
                                nc.tensor.matmul(ps[:], st[:], ths[m],
                                                 start=(m == 0),
                                                 stop=(m == 3))
                            # PSUM->SBUF copies on the scalar engine
                            nc.scalar.copy(bigA[:, o * CS:(o + 1) * CS],
                                           ps[:])
                    bufs = (lambda s: (bigA, bigB) if s % 2 == 0
                            else (bigB, bigA))
                    butterfly(nc, bufs, 8, 4 * CS, A)
                    nc.vector.tensor_reduce(am2[:, 0:1], bigB[:],
                                            axis=mybir.AxisListType.X,
                                            op=A.max,
                                            apply_absolute_value=True)
                    return bigB

                def scale_trigger(am2, red1, tag, col):
                    red = qsc.tile([128, 1], DT, tag=f"rd{tag}",
                                   name=f"rd{tag}")
                    nc.gpsimd.partition_all_reduce(
                        red[:], am2[:, 0:1], channels=128,
                        reduce_op=bass_isa.ReduceOp.absmax)
                    nc.sync.dma_start(s2_i[0:1, col:col + 1], red[0:1, 0:1])

                def scale_finish(tag, col):
                    sg = qsc.tile([1, 1], DT, tag=f"sg{tag}",
                                  name=f"sg{tag}")
                    nc.sync.dma_start(sg[0:1, :], s2_o[0:1, col:col + 1])
                    # r = QMAX/s (hardware iterative divide is accurate; a
                    # scale off by 2^-23 shifts ~no stochastic decisions)
                    r0 = qsc.tile([1, 1], DT, tag=f"r0{tag}", name=f"r0{tag}")
                    nc.vector.reciprocal(r0[0:1, :], sg[0:1, :])
                    r127 = qsc.tile([1, 1], DT, tag=f"rq{tag}",
                                    name=f"rq{tag}")
                    nc.vector.tensor_scalar_mul(r127[0:1, :], r0[0:1, :],
                                                QMAX)
                    rb = qsc.tile([128, 1], DT, tag=f"rb{tag}",
                                  name=f"rb{tag}")
                    nc.gpsimd.partition_broadcast(rb[:, 0:1], r127[0:1, 0:1])
                    return sg, rb

                def pre_transpose(big, ntiles, side):
                    """PE-transpose the rotated fp16 data [128, ntiles*CS]
                    into two k-half tiles [128, ntiles*128]; 4 blocks batch
                    into one PSUM tile so evacuation is 4x cheaper."""
                    outs = [qTp.tile([128, ntiles * 128], FH, tag="qT",
                                     name=f"{side}T{h}", bufs=4)
                            for h in range(2)]
                    for h in range(2):
                        for o4 in range(ntiles // 4):
                            ps = fps.tile([128, 512], FH, tag="tps",
                                          name="tpst", bufs=4)
                            for j in range(4):
                                o = o4 * 4 + j
                                nc.tensor.transpose(
                                    ps[:, j * 128:(j + 1) * 128],
                                    big[:, o * CS + h * 128:o * CS +
                                        (h + 1) * 128], idb[:])
                            nc.scalar.copy(
                                outs[h][:, o4 * 512:(o4 + 1) * 512], ps[:])
                    return outs

                def quant_half(tT, rb, nz, side):
                    """stochastic-round one k-half [128, N] in final layout:
                    STT -> int8 (rint via cast)."""
                    n = tT.shape[1]
                    qh = qtmp.tile([128, n], I8, tag="qh", name="qht",
                                   bufs=4)
                    nc.vector.scalar_tensor_tensor(
                        qh[:], tT[:], rb[:, 0:1], nz[:], op0=A.mult,
                        op1=A.add)
                    return qh

                # ---- w side first: fwd + AR-w + quant + A2A ----
                nzw = [qtmp.tile([128, F], FH, tag="nzw", name=f"nzw{h}",
                                 bufs=2) for h in range(2)]
                for h in range(2):
                    nc.scalar.dma_start(nzw[h][:],
                                        mk[h * 128:(h + 1) * 128, :])
                wkg = wk.ap().rearrange("(g m p) c -> g p m c",
                                        g=8, m=4)
                wrB = fwd_side(lambda g: wkg[g], F // 128, "w")
                scale_trigger(fwd_side.am2, fwd_side.red1, "w", 1)

                nzx = [qtmp.tile([128, B], FH, tag="nzx", name=f"nzx{h}",
                                 bufs=2) for h in range(2)]
                for h in range(2):
                    nc.scalar.dma_start(nzx[h][:],
                                        nk[h * 128:(h + 1) * 128, :])

                # ---- x side fwd (DVE butterfly overlaps AR-w flight);
                # emitted before pre_transpose(w) so the x H128 matmuls
                # aren't stuck on the PE FIFO behind transposes that wait
                # for the w butterfly ----
                xkg = xk.ap().rearrange("(g m p) c -> g p m c",
                                        g=8, m=4)
                xrB = fwd_side(lambda g: xkg[g], BT, "x")
                scale_trigger(fwd_side.am2, fwd_side.red1, "x", 0)
                # ONE AllReduce for both scales (saves a ~20us stream slot)
                nc.gpsimd.collective_compute(
                    "AllReduce", A.max, replica_groups=rg,
                    ins=[s2_i.ap().opt()], outs=[s2_o.ap().opt()])

                wrT = pre_transpose(wrB, F // 128, "w")
                xrT = pre_transpose(xrB, BT, "x")

                # x quant -> AG1 (evens = k-half 0) first on the stream
                sgx, rbx = scale_finish("x", 0)
                sgw, rbw = scale_finish("w", 1)
                qh0 = quant_half(xrT[0], rbx, nzx[0], "x")
                nc.sync.dma_start(xqc0[:, :], qh0[:])
                nc.gpsimd.collective_compute(
                    "AllGather", A.bypass, replica_groups=rg,
                    ins=[xqc0.ap().opt()], outs=[xqg0.ap().opt()])

                # w quant -> A2A
                wqh = [quant_half(wrT[h], rbw, nzw[h], "w")
                       for h in range(2)]
                wacr = wac.ap().rearrange("(a hh p) f -> hh p a f",
                                          a=NCORES, hh=2)
                for h in range(2):
                    nc.sync.dma_start(
                        wacr[h],
                        wqh[h][:].rearrange("p (a f) -> p a f", a=NCORES))
                nc.gpsimd.collective_compute(
                    "AllToAll", A.bypass, replica_groups=rg,
                    ins=[wac.ap().opt()], outs=[wblk.ap().opt()])

                # x odds -> AG2
                qh1 = quant_half(xrT[1], rbx, nzx[1], "x")
                nc.sync.dma_start(xqc1[:, :], qh1[:])
                nc.gpsimd.collective_compute(
                    "AllGather", A.bypass, replica_groups=rg,
                    ins=[xqc1.ap().opt()], outs=[xqg1.ap().opt()])

                # alpha = sx*sw/(QMAX^2 * 2^24)
                al = qsc.tile([1, 1], DT, tag="al", name="al")
                nc.vector.tensor_tensor(al[0:1, 0:1], sgx[0:1, 0:1],
                                        sgw[0:1, 0:1], op=A.mult)
                nc.vector.tensor_scalar_mul(
                    al[0:1, 0:1], al[0:1, 0:1],
                    float(1.0 / (QMAX * QMAX * (1 << 24))))
                alb = qsc.tile([128, 1], DT, tag="alb", name="alb")
                nc.gpsimd.partition_broadcast(alb[:, 0:1], al[0:1, 0:1])

            # ================= GEMM + fused inverse =================
            with tc.tile_pool(name="gem", bufs=1) as gem, \
                 tc.tile_pool(name="g8", bufs=2) as g8, \
                 tc.tile_pool(name="gps", bufs=1, space="PSUM") as gps, \
                 tc.tile_pool(name="inv", bufs=1) as invp:
                # int8 staging rotates; fp16 tiles persist through the GEMM
                xs = [gem.tile([128, B], FH, tag="xs", name=f"xst{kt}",
                               bufs=KT) for kt in range(KT)]
                ws_all = gem.tile([128, KT * FS], FH, tag="ws", name="ws_all")
                ws = [ws_all[:, kt * FS:(kt + 1) * FS] for kt in range(KT)]
                wblkr = wblk.ap().rearrange("(g p) f -> p g f", g=KT)
                for j in range(NCORES):   # xs evens: first on every FIFO
                    kt = 2 * j
                    x8 = g8.tile([128, B], I8, tag="x8", name=f"x8_{kt}",
                                 bufs=2)
                    nc.sync.dma_start(x8[:], xqg0[j * 128:(j + 1) * 128, :])
                    if j % 2 == 0:
                        nc.scalar.copy(xs[kt][:], x8[:])
                    else:
                        nc.vector.tensor_copy(xs[kt][:], x8[:])
                for h in range(2):        # ws: vector converts (behind A2A)
                    w8 = g8.tile([128, B], I8, tag="x8", name=f"w8_{h}",
                                 bufs=2)
                    nc.scalar.dma_start(
                        w8[:].rearrange("p (g f) -> p g f", g=8),
                        wblkr[:, h * 8:(h + 1) * 8, :])
                    nc.vector.tensor_copy(
                        ws_all[:, h * 4096:(h + 1) * 4096], w8[:])
                for j in range(NCORES):   # xs odds: vector converts (AG2)
                    kt = 2 * j + 1
                    x8 = g8.tile([128, B], I8, tag="x8", name=f"x8_{kt}",
                                 bufs=2)
                    nc.sync.dma_start(x8[:], xqg1[j * 128:(j + 1) * 128, :])
                    nc.vector.tensor_copy(xs[kt][:], x8[:])

                # evens-round stash: alpha-scaled fp16 partials [128,32*512]
                stash = invp.tile([128, 32 * FS], FH, tag="stash",
                                  name="stash")
                for g in range(8):
                    pss = [gps.tile([128, FS], DT, tag="gp",
                                    name=f"gpe{g}_{i}", bufs=4)
                           for i in range(4)]
                    for kt in range(0, KT, 2):
                        for i in range(4):
                            bo = g * 4 + i
                            nc.tensor.matmul(
                                pss[i][:],
                                xs[kt][:, bo * 128:(bo + 1) * 128],
                                ws[kt], start=(kt == 0), stop=(kt == 14))
                    for i in range(4):
                        bo = g * 4 + i
                        nc.scalar.activation(
                            stash[:, bo * FS:(bo + 1) * FS], pss[i][:],
                            mybir.ActivationFunctionType.Copy,
                            scale=alb[:, 0:1])

                # odds round + inverse pipeline per group
                for g in range(8):
                    pss = [gps.tile([128, FS], DT, tag="gp",
                                    name=f"gpo{g}_{i}", bufs=4)
                           for i in range(4)]
                    for kt in range(1, KT, 2):
                        for i in range(4):
                            bo = g * 4 + i
                            nc.tensor.matmul(
                                pss[i][:],
                                xs[kt][:, bo * 128:(bo + 1) * 128],
                                ws[kt], start=(kt == 1), stop=(kt == 15))
                    # yr = alpha*psum + stash  (fused on DVE, fp16 out)
                    yro = invp.tile([128, 4 * FS], FH, tag="yro", name="yro",
                                    bufs=2)
                    for i in range(4):
                        bo = g * 4 + i
                        nc.vector.scalar_tensor_tensor(
                            yro[:, i * FS:(i + 1) * FS], pss[i][:],
                            alb[:, 0:1], stash[:, bo * FS:(bo + 1) * FS],
                            op0=A.mult, op1=A.add)
                    # fused transpose: [b,f] -> [f,b'] with H128 on b
                    uRA = invp.tile([128, 4 * 128 * 4], FH, tag="uRA",
                                    name="uRA", bufs=1)
                    uRB = invp.tile([128, 4 * 128 * 4], FH, tag="uRB",
                                    name="uRB", bufs=1)
                    for i in range(4):
                        psT = gps.tile([128, 512], DT, tag="tp",
                                       name=f"tpt{g}_{i}", bufs=2)
                        for ft in range(4):
                            nc.tensor.matmul(
                                psT[:, ft * 128:(ft + 1) * 128],
                                yro[:, i * FS + ft * 128:
                                    i * FS + (ft + 1) * 128],
                                h128b[:], start=True, stop=True)
                        nc.scalar.copy(uRA[:, i * 512:(i + 1) * 512],
                                       psT[:])
                    # batch-H4 (bo bits 0,1; distance 1,2 in i) on the roll:
                    # layout uRA = [bo-local 4][ft 4][b' 128]
                    for h in range(2):
                        a0, a1 = h * 1024, h * 1024 + 512
                        nc.vector.tensor_tensor(uRB[:, a0:a0 + 512],
                                                uRA[:, a0:a0 + 512],
                                                uRA[:, a1:a1 + 512],
                                                op=A.add)
                        nc.vector.tensor_tensor(uRB[:, a1:a1 + 512],
                                                uRA[:, a0:a0 + 512],
                                                uRA[:, a1:a1 + 512],
                                                op=A.subtract)
                    for h in range(2):
                        a0, a1 = h * 512, h * 512 + 1024
                        nc.vector.tensor_tensor(uRA[:, a0:a0 + 512],
                                                uRB[:, a0:a0 + 512],
                                                uRB[:, a1:a1 + 512],
                                                op=A.add)
                        nc.vector.tensor_tensor(uRA[:, a1:a1 + 512],
                                                uRB[:, a0:a0 + 512],
                                                uRB[:, a1:a1 + 512],
                                                op=A.subtract)
                    # feature H128 per out f-tile; H32/H8 fold into host
                    for ft in range(4):
                        zps = gps.tile([128, 512], DT, tag="zp",
                                       name=f"zps{g}_{ft}", bufs=2)
                        nc.tensor.matmul(
                            zps[:],
                            h128b[:],
                            uRA[:].rearrange("p (bo f b) -> p f bo b",
                                             bo=4, f=4)[:, ft, :, :],
                            start=True, stop=True)
                        ostg = invp.tile([128, 512], FH, tag="ostg",
                                         name=f"ostg{g}_{ft}", bufs=2)
                        nc.scalar.copy(ostg[:], zps[:])
                        nc.sync.dma_start(
                            out[ft * 128:(ft + 1) * 128,
                                g * 512:(g + 1) * 512], ostg[:])
            qsc.release()
    nc.compile()
    return nc


def kernel(**inputs):
    from concourse.bass_utils import run_bass_kernel_spmd

    if "nc" not in _cache:
        _cache["nc"] = _build()
    nc = _cache["nc"]

    x = np.asarray(inputs["inputs"], np.float32).astype(np.float16)
    w = np.asarray(inputs["kernel"], np.float32).astype(np.float16)
    bias = np.asarray(inputs["bias"], np.float32)
    nxp = (0.5 - np.asarray(inputs["noise_x"], np.float32)).astype(np.float16)
    nwp = (0.5 - np.asarray(inputs["noise_w"], np.float32)).astype(np.float16)

    in_maps = []
    for k in range(NCORES):
        cs = slice(k * CS, (k + 1) * CS)
        in_maps.append({
            "xk": np.ascontiguousarray(x[:, cs]),
            "nk": np.ascontiguousarray(nxp[:, cs].T),
            "wk": np.ascontiguousarray(w[cs, :].T),
            "mk": np.ascontiguousarray(nwp[cs, :]),
        })

    res = run_bass_kernel_spmd(nc, in_maps, list(range(NCORES)))
    V = np.stack([np.asarray(r["out"], np.float32)
                  for r in res.results])                   # [a, 4ft*128, B]
    H32 = _sylvester(32)
    H8 = _sylvester(8)
    yT = (H32 @ V.reshape(NCORES * 4, -1)).reshape(F, 8, 512)  # feature H32
    yT = np.einsum('gc,fcb->fgb', H8, yT).reshape(F, B)        # batch H8
    y = np.ascontiguousarray(yT.T) + bias[None, :]
    return y.astype(np.float32)


# revision 15
# speedup vs baseline: 1.2664x; 1.2664x over previous
"""Trainium2 Bass kernel for quantized dense layer with Hadamard rotations.

Math (see reference): y = (H2 @ (sq(H2@x) @ sq(w@H1)) @ H1)/(64*64) + bias,
where sq() is per-tensor symmetric int8 stochastic quantization.

Structure: Sylvester Hadamards factor as Kronecker products
(H4096 = H32 (x) H128).  The forward transform per side is a per-128-tile
fp16 PE matmul against H128 (inputs converted fp32->fp16; validated to
cause ~1.2% stochastic-rounding flips = ~0.45% operand error) plus a
cross-tile DVE butterfly in fp16.  Quantized values (<=127) are exact in
fp16, so the core GEMM runs fp16 x fp16 -> fp32 PSUM exactly.  Stochastic
rounding is rint(x*scale + (0.5 - noise)) via the fp32->int16
round-to-nearest cast, narrowed to int8 for the collectives.

Sharding (8 cores): the IN axis is split 8 ways for forward transforms +
quantization.  v2 schedule: the W side runs FIRST so its scale AllReduce
and the AllToAll land early; the CC stream order is
warmup-AR, AR-w, AR-x, A2A(w int8), AG1(x evens int8), AG2(x odds int8),
so the core GEMM starts as soon as AG1 lands instead of waiting for the
whole chain.  All data collectives ship int8 (half the bytes of fp16);
int8->fp16 conversion rides the scalar/vector engines during the
PE-bound GEMM phase.

The GEMM runs in two rounds: evens k-tiles accumulate while AG2 is in
flight and are stashed to SBUF as alpha-scaled fp16; the odds round adds
the stash back via a fused scalar_tensor_tensor.  The inverse fuses the
batch H128 into the post-GEMM PE transpose and applies the batch H4
(bo bits 0-1) as a DVE roll; the feature H128 is one PE matmul per
f-tile.  The remaining inverse factors (feature H32 over core x f-tile,
batch H8 over bo-chunk) fold into the host-side unshard combine.

Known hardware behaviors factored in: PE HAM throttle (1.2 GHz cold /
2.4 GHz after ~3.4us sustained); fp32 matmuls lower to 2 half-speed
passes -- avoid; collectives serialize on one CC stream (emission order
= stream order) with ~10-30us latency floors; a warmup AllReduce at t=0
absorbs the CC-entry barrier and inter-core launch skew; scalar-engine
copies offload PSUM evacuation.
"""
import sys, os
sys.path.insert(0, '/opt/trn_rl_repo')
import numpy as np

B, IN, F = 4096, 2048, 4096
NCORES = 8
CS = IN // NCORES      # 256  per-core IN slice
FS = F // NCORES       # 512  per-core feature block
BT = B // 128          # 32   batch tiles
KT = IN // 128         # 16   contraction tiles
QMAX = 127.0

_cache = {}


def _sylvester(n):
    h = np.array([[1.0]], dtype=np.float32)
    while h.shape[0] < n:
        h = np.block([[h, h], [h, -h]])
    return h


def _build():
    from concourse import bass, bacc, tile, mybir
    import concourse.bass_isa as bass_isa

    DT = mybir.dt.float32
    FH = mybir.dt.float16
    I16 = mybir.dt.int16
    I8 = mybir.dt.int8
    A = mybir.AluOpType
    nph = np.float16

    nc = bacc.Bacc("TRN2", target_bir_lowering=False, debug=False,
                   num_devices=NCORES)

    xk = nc.dram_tensor("xk", [B, CS], FH, kind="ExternalInput")
    nk = nc.dram_tensor("nk", [CS, B], FH, kind="ExternalInput")   # (0.5-noise_x)^T
    wk = nc.dram_tensor("wk", [F, CS], FH, kind="ExternalInput")   # w slice^T
    mk = nc.dram_tensor("mk", [CS, F], FH, kind="ExternalInput")   # 0.5-noise_w
    out = nc.dram_tensor("out", [FS, B], FH, kind="ExternalOutput")

    wu_o = nc.dram_tensor("wu_o", [1, 1], DT, addr_space="Shared")
    s2_i = nc.dram_tensor("s2_i", [1, 2], DT)
    s2_o = nc.dram_tensor("s2_o", [1, 2], DT, addr_space="Shared")
    xqc0 = nc.dram_tensor("xqc0", [128, B], I8)                    # xq^T k-half 0
    xqc1 = nc.dram_tensor("xqc1", [128, B], I8)                    # xq^T k-half 1
    xqg0 = nc.dram_tensor("xqg0", [NCORES * 128, B], I8, addr_space="Shared")
    xqg1 = nc.dram_tensor("xqg1", [NCORES * 128, B], I8, addr_space="Shared")
    wac = nc.dram_tensor("wac", [IN, FS], I8)                      # A2A contrib
    wblk = nc.dram_tensor("wblk", [IN, FS], I8)

    wu_i = nc.inline_tensor(np.zeros((1, 1), np.float32), name="wu_i")
    h128b_d = nc.inline_tensor(_sylvester(128).astype(nph), name="h128b")
    h128n_d = nc.inline_tensor((-_sylvester(128)).astype(nph), name="h128n")
    idb_d = nc.inline_tensor(np.eye(128, dtype=nph), name="idb")
    H4 = _sylvester(4)
    rg = [list(range(NCORES))]

    NB = 32 * CS  # 8192 free columns in a fwd big tile

    def butterfly(nc, bufs, T, blk0, A):
        """FWHT across the tile-index axis of big tensors [128, T*blk0]."""
        n = T.bit_length() - 1
        for s in range(n):
            cur, nxt = bufs(s)
            blk = blk0 << s
            hi = T >> (s + 1)
            for h in range(hi):
                a0 = h * 2 * blk
                a1 = a0 + blk
                nc.vector.tensor_tensor(nxt[:, a0:a0 + blk],
                                        cur[:, a0:a0 + blk],
                                        cur[:, a1:a1 + blk], op=A.add)
                nc.vector.tensor_tensor(nxt[:, a1:a1 + blk],
                                        cur[:, a0:a0 + blk],
                                        cur[:, a1:a1 + blk],
                                        op=A.subtract)

    with tile.TileContext(nc) as tc:
        with tc.tile_pool(name="consts", bufs=1) as cpool:
            h128b = cpool.tile([128, 128], FH)
            h128n = cpool.tile([128, 128], FH)
            idb = cpool.tile([128, 128], FH)
            nc.sync.dma_start(h128b[:], h128b_d[:])
            nc.sync.dma_start(h128n[:], h128n_d[:])
            nc.sync.dma_start(idb[:], idb_d[:])
            qsc = tc.alloc_tile_pool(name="qsc", bufs=1)
            nc.gpsimd.collective_compute(
                "AllReduce", A.max, replica_groups=rg,
                ins=[wu_i.ap().opt()], outs=[wu_o.ap().opt()])

            # ================= forward transforms + quant =================
            with tc.tile_pool(name="fwd", bufs=2) as fp_, \
                 tc.tile_pool(name="fin", bufs=4) as fin, \
                 tc.tile_pool(name="fps", bufs=1, space="PSUM") as fps, \
                 tc.tile_pool(name="qtmp", bufs=2) as qtmp, \
                 tc.tile_pool(name="qT", bufs=3) as qTp:

                def fwd_side(src_tile_ap, ntiles, side):
                    am2 = qsc.tile([128, 2], DT, tag=f"am{side}",
                                   name=f"am{side}")
                    fwd_side.am2 = am2
                    fwd_side.red1 = qsc.tile([1, 1], DT, tag=f"r1{side}",
                                             name=f"r1{side}")
                    bigA = fp_.tile([128, NB], FH, tag="bigA",
                                    name=f"bigA{side}")
                    bigB = fp_.tile([128, NB], FH, tag="bigB",
                                    name=f"bigB{side}")
                    # H128 (x) H4: per 4-tile group, each output tile is a
                    # 4-term +/-H128 PSUM accumulation (DVE TT runs at 1x
                    # mode, so trading 2 butterfly stages for PE matmuls
                    # wins; the PE load also warms the HAM clock early)
                    for g4 in range(ntiles // 4):
                        thg = fin.tile([128, 4 * CS], FH, tag="finh",
                                       name="finth", bufs=4)
                        nc.sync.dma_start(
                            thg[:].rearrange("p (m c) -> p m c", m=4),
                            src_tile_ap(g4))
                        ths = [thg[:, m * CS:(m + 1) * CS] for m in range(4)]
                        for mp in range(4):
                            o = g4 * 4 + mp
                            ps = fps.tile([128, CS], DT, tag="ps",
                                          name="fpst", bufs=4)
                            for m in range(4):
                                st = h128b if H4[mp, m] > 0 else h128n
                                nc.tensor.matmul(ps[:], st[:], ths[m],
                                                 start=(m == 0),
                                                 stop=(m == 3))
                            # PSUM->SBUF copies on the scalar engine
                            nc.scalar.copy(bigA[:, o * CS:(o + 1) * CS],
                                           ps[:])
                    bufs = (lambda s: (bigA, bigB) if s % 2 == 0
                            else (bigB, bigA))
                    butterfly(nc, bufs, 8, 4 * CS, A)
                    nc.vector.tensor_reduce(am2[:, 0:1], bigB[:],
                                            axis=mybir.AxisListType.X,
                                            op=A.max,
                                            apply_absolute_value=True)
                    return bigB

                def scale_trigger(am2, red1, tag, col):
                    red = qsc.tile([128, 1], DT, tag=f"rd{tag}",
                                   name=f"rd{tag}")
                    nc.gpsimd.partition_all_reduce(
                        red[:], am2[:, 0:1], channels=128,
                        reduce_op=bass_isa.ReduceOp.absmax)
                    nc.sync.dma_start(s2_i[0:1, col:col + 1], red[0:1, 0:1])

                def scale_finish(tag, col):
                    sg = qsc.tile([1, 1], DT, tag=f"sg{tag}",
                                  name=f"sg{tag}")
                    nc.sync.dma_start(sg[0:1, :], s2_o[0:1, col:col + 1])
                    # r = QMAX/s (hardware iterative divide is accurate; a
                    # scale off by 2^-23 shifts ~no stochastic decisions)
                    r0 = qsc.tile([1, 1], DT, tag=f"r0{tag}", name=f"r0{tag}")
                    nc.vector.reciprocal(r0[0:1, :], sg[0:1, :])
                    r127 = qsc.tile([1, 1], DT, tag=f"rq{tag}",
                                    name=f"rq{tag}")
                    nc.vector.tensor_scalar_mul(r127[0:1, :], r0[0:1, :],
                                                QMAX)
                    rb = qsc.tile([128, 1], DT, tag=f"rb{tag}",
                                  name=f"rb{tag}")
                    nc.gpsimd.partition_broadcast(rb[:, 0:1], r127[0:1, 0:1])
                    return sg, rb

                def pre_transpose(big, ntiles, side):
                    """PE-transpose the rotated fp16 data [128, ntiles*CS]
                    into two k-half tiles [128, ntiles*128]; 4 blocks batch
                    into one PSUM tile so evacuation is 4x cheaper."""
                    outs = [qTp.tile([128, ntiles * 128], FH, tag="qT",
                                     name=f"{side}T{h}", bufs=4)
                            for h in range(2)]
                    for h in range(2):
                        for o4 in range(ntiles // 4):
                            ps = fps.tile([128, 512], FH, tag="tps",
                                          name="tpst", bufs=4)
                            for j in range(4):
                                o = o4 * 4 + j
                                nc.tensor.transpose(
                                    ps[:, j * 128:(j + 1) * 128],
                                    big[:, o * CS + h * 128:o * CS +
                                        (h + 1) * 128], idb[:])
                            nc.scalar.copy(
                                outs[h][:, o4 * 512:(o4 + 1) * 512], ps[:])
                    return outs

                def quant_half(tT, rb, nz, side):
                    """stochastic-round one k-half [128, N] in final layout:
                    STT -> int8 (rint via cast)."""
                    n = tT.shape[1]
                    qh = qtmp.tile([128, n], I8, tag="qh", name="qht",
                                   bufs=4)
                    nc.vector.scalar_tensor_tensor(
                        qh[:], tT[:], rb[:, 0:1], nz[:], op0=A.mult,
                        op1=A.add)
                    return qh

                # ---- w side first: fwd + AR-w + quant + A2A ----
                nzw = [qtmp.tile([128, F], FH, tag="nzw", name=f"nzw{h}",
                                 bufs=2) for h in range(2)]
                for h in range(2):
                    nc.scalar.dma_start(nzw[h][:],
                                        mk[h * 128:(h + 1) * 128, :])
                wkg = wk.ap().rearrange("(g m p) c -> g p m c",
                                        g=8, m=4)
                wrB = fwd_side(lambda g: wkg[g], F // 128, "w")
                scale_trigger(fwd_side.am2, fwd_side.red1, "w", 1)

                nzx = [qtmp.tile([128, B], FH, tag="nzx", name=f"nzx{h}",
                                 bufs=2) for h in range(2)]
                for h in range(2):
                    nc.scalar.dma_start(nzx[h][:],
                                        nk[h * 128:(h + 1) * 128, :])

                # ---- x side fwd (DVE butterfly overlaps AR-w flight);
                # emitted before pre_transpose(w) so the x H128 matmuls
                # aren't stuck on the PE FIFO behind transposes that wait
                # for the w butterfly ----
                xkg = xk.ap().rearrange("(g m p) c -> g p m c",
                                        g=8, m=4)
                xrB = fwd_side(lambda g: xkg[g], BT, "x")
                scale_trigger(fwd_side.am2, fwd_side.red1, "x", 0)
                # ONE AllReduce for both scales (saves a ~20us stream slot)
                nc.gpsimd.collective_compute(
                    "AllReduce", A.max, replica_groups=rg,
                    ins=[s2_i.ap().opt()], outs=[s2_o.ap().opt()])

                wrT = pre_transpose(wrB, F // 128, "w")
                xrT = pre_transpose(xrB, BT, "x")

                # x quant -> AG1 (evens = k-half 0) first on the stream
                sgx, rbx = scale_finish("x", 0)
                sgw, rbw = scale_finish("w", 1)
                qh0 = quant_half(xrT[0], rbx, nzx[0], "x")
                nc.sync.dma_start(xqc0[:, :], qh0[:])
                nc.gpsimd.collective_compute(
                    "AllGather", A.bypass, replica_groups=rg,
                    ins=[xqc0.ap().opt()], outs=[xqg0.ap().opt()])

                # w quant -> A2A
                wqh = [quant_half(wrT[h], rbw, nzw[h], "w")
                       for h in range(2)]
                wacr = wac.ap().rearrange("(a hh p) f -> hh p a f",
                                          a=NCORES, hh=2)
                for h in range(2):
                    nc.sync.dma_start(
                        wacr[h],
                        wqh[h][:].rearrange("p (a f) -> p a f", a=NCORES))
                nc.gpsimd.collective_compute(
                    "AllToAll", A.bypass, replica_groups=rg,
                    ins=[wac.ap().opt()], outs=[wblk.ap().opt()])

                # x odds -> AG2
                qh1 = quant_half(xrT[1], rbx, nzx[1], "x")
                nc.sync.dma_start(xqc1[:, :], qh1[:])
                nc.gpsimd.collective_compute(
                    "AllGather", A.bypass, replica_groups=rg,
                    ins=[xqc1.ap().opt()], outs=[xqg1.ap().opt()])

                # alpha = sx*sw/(QMAX^2 * 2^24)
                al = qsc.tile([1, 1], DT, tag="al", name="al")
                nc.vector.tensor_tensor(al[0:1, 0:1], sgx[0:1, 0:1],
                                        sgw[0:1, 0:1], op=A.mult)
                nc.vector.tensor_scalar_mul(
                    al[0:1, 0:1], al[0:1, 0:1],
                    float(1.0 / (QMAX * QMAX * (1 << 24))))
                alb = qsc.tile([128, 1], DT, tag="alb", name="alb")
                nc.gpsimd.partition_broadcast(alb[:, 0:1], al[0:1, 0:1])

            # ================= GEMM + fused inverse =================
            with tc.tile_pool(name="gem", bufs=1) as gem, \
                 tc.tile_pool(name="g8", bufs=2) as g8, \
                 tc.tile_pool(name="gps", bufs=1, space="PSUM") as gps, \
                 tc.tile_pool(name="inv", bufs=1) as invp:
                # int8 staging rotates; fp16 tiles persist through the GEMM
                xs = [gem.tile([128, B], FH, tag="xs", name=f"xst{kt}",
                               bufs=KT) for kt in range(KT)]
                ws_all = gem.tile([128, KT * FS], FH, tag="ws", name="ws_all")
                ws = [ws_all[:, kt * FS:(kt + 1) * FS] for kt in range(KT)]
                wblkr = wblk.ap().rearrange("(g p) f -> p g f", g=KT)
                for j in range(NCORES):   # xs evens: first on every FIFO
                    kt = 2 * j
                    x8 = g8.tile([128, B], I8, tag="x8", name=f"x8_{kt}",
                                 bufs=2)
                    nc.sync.dma_start(x8[:], xqg0[j * 128:(j + 1) * 128, :])
                    if j % 2 == 0:
                        nc.scalar.copy(xs[kt][:], x8[:])
                    else:
                        nc.vector.tensor_copy(xs[kt][:], x8[:])
                for h in range(2):        # ws: vector converts (behind A2A)
                    w8 = g8.tile([128, B], I8, tag="x8", name=f"w8_{h}",
                                 bufs=2)
                    nc.scalar.dma_start(
                        w8[:].rearrange("p (g f) -> p g f", g=8),
                        wblkr[:, h * 8:(h + 1) * 8, :])
                    nc.vector.tensor_copy(
                        ws_all[:, h * 4096:(h + 1) * 4096], w8[:])
                for j in range(NCORES):   # xs odds: vector converts (AG2)
                    kt = 2 * j + 1
                    x8 = g8.tile([128, B], I8, tag="x8", name=f"x8_{kt}",
                                 bufs=2)
                    nc.sync.dma_start(x8[:], xqg1[j * 128:(j + 1) * 128, :])
                    nc.vector.tensor_copy(xs[kt][:], x8[:])

                # evens-round stash: alpha-scaled fp16 partials [128,32*512]
                stash = invp.tile([128, 32 * FS], FH, tag="stash",
                                  name="stash")
                for g in range(8):
                    pss = [gps.tile([128, FS], DT, tag="gp",
                                    name=f"gpe{g}_{i}", bufs=4)
                           for i in range(4)]
                    for kt in range(0, KT, 2):
                        for i in range(4):
                            bo = g * 4 + i
                            nc.tensor.matmul(
                                pss[i][:],
                                xs[kt][:, bo * 128:(bo + 1) * 128],
                                ws[kt], start=(kt == 0), stop=(kt == 14))
                    for i in range(4):
                        bo = g * 4 + i
                        nc.scalar.activation(
                            stash[:, bo * FS:(bo + 1) * FS], pss[i][:],
                            mybir.ActivationFunctionType.Copy,
                            scale=alb[:, 0:1])

                # odds round + inverse pipeline per group
                for g in range(8):
                    pss = [gps.tile([128, FS], DT, tag="gp",
                                    name=f"gpo{g}_{i}", bufs=4)
                           for i in range(4)]
                    for kt in range(1, KT, 2):
                        for i in range(4):
                            bo = g * 4 + i
                            nc.tensor.matmul(
                                pss[i][:],
                                xs[kt][:, bo * 128:(bo + 1) * 128],
                                ws[kt], start=(kt == 1), stop=(kt == 15))
                    # yr = alpha*psum + stash  (fused on DVE, fp16 out)
                    yro = invp.tile([128, 4 * FS], FH, tag="yro", name="yro",
                                    bufs=2)
                    for i in range(4):
                        bo = g * 4 + i
                        nc.vector.scalar_tensor_tensor(
                            yro[:, i * FS:(i + 1) * FS], pss[i][:],
                            alb[:, 0:1], stash[:, bo * FS:(bo + 1) * FS],
                            op0=A.mult, op1=A.add)
                    # fused transpose: [b,f] -> [f,b'] with H128 on b
                    uRA = invp.tile([128, 4 * 128 * 4], FH, tag="uRA",
                                    name="uRA", bufs=1)
                    uRB = invp.tile([128, 4 * 128 * 4], FH, tag="uRB",
                                    name="uRB", bufs=1)
                    for i in range(4):
                        psT = gps.tile([128, 512], DT, tag="tp",
                                       name=f"tpt{g}_{i}", bufs=2)
                        for ft in range(4):
                            nc.tensor.matmul(
                                psT[:, ft * 128:(ft + 1) * 128],
                                yro[:, i * FS + ft * 128:
                                    i * FS + (ft + 1) * 128],
                                h128b[:], start=True, stop=True)
                        nc.scalar.copy(uRA[:, i * 512:(i + 1) * 512],
                                       psT[:])
                    # batch-H4 (bo bits 0,1; distance 1,2 in i) on the roll:
                    # layout uRA = [bo-local 4][ft 4][b' 128]
                    for h in range(2):
                        a0, a1 = h * 1024, h * 1024 + 512
                        nc.vector.tensor_tensor(uRB[:, a0:a0 + 512],
                                                uRA[:, a0:a0 + 512],
                                                uRA[:, a1:a1 + 512],
                                                op=A.add)
                        nc.vector.tensor_tensor(uRB[:, a1:a1 + 512],
                                                uRA[:, a0:a0 + 512],
                                                uRA[:, a1:a1 + 512],
                                                op=A.subtract)
                    for h in range(2):
                        a0, a1 = h * 512, h * 512 + 1024
                        nc.vector.tensor_tensor(uRA[:, a0:a0 + 512],
                                                uRB[:, a0:a0 + 512],
                                                uRB[:, a1:a1 + 512],
                                                op=A.add)
                        nc.vector.tensor_tensor(uRA[:, a1:a1 + 512],
                                                uRB[:, a0:a0 + 512],
                                                uRB[:, a1:a1 + 512],
                                                op=A.subtract)
                    # feature H128 per out f-tile; H32/H8 fold into host
                    for ft in range(4):
                        zps = gps.tile([128, 512], DT, tag="zp",
                                       name=f"zps{g}_{ft}", bufs=2)
                        nc.tensor.matmul(
                            zps[:],
                            h128b[:],
                            uRA[:].rearrange("p (bo f b) -> p f bo b",
                                             bo=4, f=4)[:, ft, :, :],
                            start=True, stop=True)
                        ostg = invp.tile([128, 512], FH, tag="ostg",
                                         name=f"ostg{g}_{ft}", bufs=2)
                        nc.scalar.copy(ostg[:], zps[:])
                        nc.sync.dma_start(
                            out[ft * 128:(ft + 1) * 128,
                                g * 512:(g + 1) * 512], ostg[:])
            qsc.release()
    nc.compile()
    return nc


def kernel(**inputs):
    from concourse.bass_utils import run_bass_kernel_spmd

    if "nc" not in _cache:
        _cache["nc"] = _build()
    nc = _cache["nc"]

    x = np.asarray(inputs["inputs"], np.float32).astype(np.float16)
    w = np.asarray(inputs["kernel"], np.float32).astype(np.float16)
    bias = np.asarray(inputs["bias"], np.float32)
    nxp = (0.5 - np.asarray(inputs["noise_x"], np.float32)).astype(np.float16)
    nwp = (0.5 - np.asarray(inputs["noise_w"], np.float32)).astype(np.float16)

    in_maps = []
    for k in range(NCORES):
        cs = slice(k * CS, (k + 1) * CS)
        in_maps.append({
            "xk": np.ascontiguousarray(x[:, cs]),
            "nk": np.ascontiguousarray(nxp[:, cs].T),
            "wk": np.ascontiguousarray(w[cs, :].T),
            "mk": np.ascontiguousarray(nwp[cs, :]),
        })

    res = run_bass_kernel_spmd(nc, in_maps, list(range(NCORES)))
    V = np.stack([np.asarray(r["out"], np.float32)
                  for r in res.results])                   # [a, 4ft*128, B]
    H32 = _sylvester(32)
    H8 = _sylvester(8)
    yT = (H32 @ V.reshape(NCORES * 4, -1)).reshape(F, 8, 512)  # feature H32
    yT = np.einsum('gc,fcb->fgb', H8, yT).reshape(F, B)        # batch H8
    y = np.ascontiguousarray(yT.T) + bias[None, :]
    return y.astype(np.float32)
